# revision 22
# baseline (speedup 1.0000x reference)
"""DiffJPEG Trainium2 Bass kernel (self-contained).

Pure data-parallel over 8 NeuronCores (4 images each). Per image the pipeline
is four matmul stages in a ds/std/ds/std chain (ds = data-stationary: image
data rides the PE stationary operand; std = constant-stationary weights, data
streams as a wide rhs):

  S1 ds  [row,col] -> [col,(I,u)] : vertical DCT (+RGB->YCC fold, 2x1 avg)
       Y and chroma accumulate into SEPARATE psum banks laid out so each
       evicts with ONE contiguous [128,512] ACT copy (no strided shuffles).
  S2 std -> [(J,v),(I,u)]         : horizontal DCT (+1/fq fold; 1x2 avg)
  quant: one [128,3072] f32 tile per image holds all Y+C coefficients; the
       diff-round runs as SIX wide passes (q, t1=q+MAGIC [DVE 2x_2p],
       dp [DVE 1x], d2 [ACT square], g [DVE 2x], r [DVE 1x], r2=r*T
       [DVE 2x_1p vs a materialized full T tile]).
  S3 ds  -> [(I,u),col]           : horizontal iDCT (+fq fold). Chroma stays
       at HALF width (no horizontal upsample here) - S4 reads it through a
       step-0 column-repeat AP, halving S3 chroma PE+eviction work.
  S4 std -> [(I,x),col]           : vertical iDCT + YCC->RGB folded into PSUM
       accumulation (chroma weights pre-scaled; vertical 2x upsample folded
       in w4cs). Output clamp is split: ACT evicts with Relu (lower bound),
       DVE applies min(x,1) in 4x mode on the fp16 tile.

Precision: encode side fp16 (bf16 flips quantizer rounding decisions);
decode side fp16 throughout (more mantissa than bf16; all decode values are
well inside fp16 range). Pixels are host-centered by -128/255 so color rows
annihilate the DC offset; decode +128 rides the S3 Y eviction as an ACT bias
on u==0 partitions. Images are two-deep software-pipelined so engines stay
busy across stage boundaries. GPSIMD/Pool deliberately unused (real-HW cost
far exceeds the model).
"""
import sys
import numpy as np

sys.path.insert(0, "/opt/trn_rl_repo")

import ml_dtypes

F16 = np.float16
N_CORES = 8
IMGS = 4          # images per core
H = W = 512
MAGIC = 1536.0  # 1.5*2**10: f16(x+M)-M == round-half-even(x) for |x|<512
# (|q| = |coeff/(T*fq)| <= ~320 here; the f32->f16 double rounding flips
# ~1e-4 of round decisions, adding ~3e-3 rel err - well inside tolerance,
# and it makes the whole correction chain 16-bit so DVE runs in 2x mode)

# ---------------------------------------------------------------------------
# host-side constants
# ---------------------------------------------------------------------------
_xs = np.arange(8, dtype=np.float32)
_COS = np.cos((2 * _xs[:, None] + 1) * _xs[None, :] * np.pi / 16).astype(np.float32)
_alpha = np.array([1.0 / np.sqrt(2)] + [1.0] * 7, dtype=np.float32)
_Y_TABLE = np.array([
    [16, 11, 10, 16, 24, 40, 51, 61], [12, 12, 14, 19, 26, 58, 60, 55],
    [14, 13, 16, 24, 40, 57, 69, 56], [14, 17, 22, 29, 51, 87, 80, 62],
    [18, 22, 37, 56, 68, 109, 103, 77], [24, 35, 55, 64, 81, 104, 113, 92],
    [49, 64, 78, 87, 103, 121, 120, 101], [72, 92, 95, 98, 112, 100, 103, 99]],
    dtype=np.float32)
_C_TABLE = np.full((8, 8), 99.0, dtype=np.float32)
_C_TABLE[:4, :4] = np.array([[17, 18, 24, 47], [18, 21, 26, 66],
                             [24, 26, 56, 99], [47, 66, 99, 99]], dtype=np.float32)
_RGB2YCC = np.array([[0.299, 0.587, 0.114],
                    [-0.168736, -0.331264, 0.5],
                    [0.5, -0.418688, -0.081312]], dtype=np.float32)
_YCC2RGB = np.array([[1.0, 0.0, 1.402],
                    [1.0, -0.344136, -0.714136],
                    [1.0, 1.772, 0.0]], dtype=np.float32)


def _bd(M, n):
    r, c = M.shape
    out = np.zeros((r * n, c * n), dtype=np.float64)
    for i in range(n):
        out[i * r:(i + 1) * r, i * c:(i + 1) * c] = M
    return out


def _base_mats():
    Av = (_COS.astype(np.float64) * 0.5 * _alpha.astype(np.float64)[None, :])  # [x,u]
    Avi = Av.T.copy()                                   # [u,x]
    Avs = np.zeros((16, 8))                             # subsample fwd
    for x2 in range(16):
        Avs[x2] = Av[x2 // 2] / 2.0
    Avu = np.zeros((8, 16))                             # upsample inv
    for x2 in range(16):
        Avu[:, x2] = Avi[:, x2 // 2]
    return Av, Avi, Avs, Avu


def build_core_inputs(x_core, quality_core):
    """x_core [IMGS,3,512,512] f32, quality_core [IMGS] f32 -> in_map dict."""
    Av, Avi, Avs, Avu = _base_mats()
    f32 = np.float32
    bd16v = _bd(Av, 16)        # [128,128] 1D fwd (vertical or horizontal)
    bd8s = _bd(Avs, 8)         # [128,64]  fwd subsampled
    bd16i = _bd(Avi, 16)       # [128,128] 1D inverse
    bd8i = _bd(Avi, 8)         # [64,64]   inverse (no upsample)
    bd8u = _bd(Avu, 8)         # [64,128]  inverse upsampling
    bd8i2 = np.concatenate([bd8i, bd8i], axis=0)        # [128,64] parity-stacked
    bd8u2 = np.concatenate([bd8u, bd8u], axis=0)        # [128,128] parity-stacked

    # S1 rhs per plane: out cols = [Y-Iu 128 | cb-I'u 64 | cr-I'u 64]
    w1 = np.stack([
        np.concatenate([255.0 * _RGB2YCC[0, p] * bd16v,
                        255.0 * _RGB2YCC[1, p] * bd8s,
                        255.0 * _RGB2YCC[2, p] * bd8s], axis=1).astype(F16)
        for p in range(3)])                                            # [3,128,256]

    fqs = []
    for q in np.asarray(quality_core, dtype=np.float64):
        factor = (5000.0 / q if q < 50.0 else 200.0 - 2.0 * q) / 100.0
        fqs.append(factor)

    w2y = np.stack([(bd16v / fq).astype(F16) for fq in fqs])   # [4,128,128]
    w2c = np.stack([(bd8s / fq).astype(F16) for fq in fqs])    # [4,128,64]
    w3y = np.stack([(bd16i * fq).astype(F16) for fq in fqs])   # [4,128,128]
    w3c = np.stack([(bd8i2 * fq).astype(F16) for fq in fqs])   # [4,128,64]
    w4y = (bd16i / 255.0).astype(F16)                          # [128,128]
    C = _YCC2RGB.astype(np.float64)
    w4cs = np.stack([(c * bd8u2 / 255.0).astype(F16)
                     for c in (C[0, 2], C[1, 1], C[1, 2], C[2, 1])])   # [4,128,128]

    # quant patterns in [(J,v) partition, (I,u) free] layout:
    # value[p, f] = T[u(f%8), v(p%8)] -> tile T.T along partitions
    rho_y = np.tile((1.0 / _Y_TABLE).T, (16, 1)).astype(f32)           # [128,8]
    rho_c = np.tile((1.0 / _C_TABLE).T, (16, 1)).astype(f32)
    # full dequant-table tile matching the merged [128,3072] coeff layout
    tfull = np.concatenate(
        [np.tile(_Y_TABLE.T, (16, 256)), np.tile(_C_TABLE.T, (16, 128))],
        axis=1).astype(F16)                                            # [128,3072]

    mask = (np.arange(128) % 8 == 0).astype(f32)[:, None]
    # decode-side +128 on Y: bias on zy u==0 partitions through w4y's
    # Avi[0,x]/255 gain -> +0.5 on every output pixel
    zyb = (mask * (0.5 * 255.0 / float(Avi[0, 0]))).astype(f32)        # [128,1]

    # batched const blocks (one DMA each; per-DMA issue cost is ~500ns so 29
    # separate weight loads would stall the head of the kernel)
    enc = np.concatenate(
        [w1[0], w1[1], w1[2]] + [w2y[m] for m in range(IMGS)]
        + [w2c[m] for m in range(IMGS)], axis=1)                       # [128,1536]
    dec = np.concatenate(
        [w3y[m] for m in range(IMGS)] + [w3c[m] for m in range(IMGS)]
        + [w4y] + [w4cs[k] for k in range(4)] + [tfull], axis=1)       # [128,4480]
    fblk = np.concatenate([rho_y, rho_c, zyb], axis=1).astype(f32)     # [128,17]

    # centered pixels: the color rows annihilate the 128 offset exactly
    # (chroma rows sum to 0, Y row to 1), shrinking all encode magnitudes
    xc = np.ascontiguousarray(x_core, dtype=np.float32) - np.float32(128.0 / 255.0)
    return {
        "x": xc.astype(F16),
        "enc": np.ascontiguousarray(enc), "dec": np.ascontiguousarray(dec),
        "fblk": fblk,
    }


# ---------------------------------------------------------------------------
# bass program
# ---------------------------------------------------------------------------
def build_program(repeat=1):
    import concourse.bacc as bacc
    import concourse.mybir as mybir
    from concourse.tile import TileContext

    f32 = mybir.dt.float32
    f16 = mybir.dt.float16
    op = mybir.AluOpType
    AF = mybir.ActivationFunctionType

    nc = bacc.Bacc("TRN2", target_bir_lowering=False, debug=False,
                   enable_asserts=False, num_devices=N_CORES)

    x_d = nc.dram_tensor("x", [IMGS, 3, H, W], f16, kind="ExternalInput").ap()
    out_d = nc.dram_tensor("out", [IMGS, 3, H, W], f16, kind="ExternalOutput").ap()
    enc_d = nc.dram_tensor("enc", [128, 1536], f16, kind="ExternalInput").ap()
    dec_d = nc.dram_tensor("dec", [128, 4480], f16, kind="ExternalInput").ap()
    fblk_d = nc.dram_tensor("fblk", [128, 17], f32, kind="ExternalInput").ap()

    with TileContext(nc, trace_sim=False) as tc:
        with tc.tile_pool(name="consts", bufs=1) as cp, \
             tc.tile_pool(name="pix", bufs=3) as pixp, \
             tc.tile_pool(name="h1", bufs=8) as h1p, \
             tc.tile_pool(name="qq", bufs=2) as qp, \
             tc.tile_pool(name="tmp", bufs=1) as tp, \
             tc.tile_pool(name="r2", bufs=2) as r2p, \
             tc.tile_pool(name="zz", bufs=7) as zp, \
             tc.tile_pool(name="ev", bufs=3) as evp, \
             tc.tile_pool(name="outp", bufs=2) as op_, \
             tc.tile_pool(name="ps", bufs=1, space="PSUM") as pp:

            # ---- batched const loads, interleaved with the first two pixel
            # DMAs so S1 of image 0 can start ~6us in instead of ~19us ----
            enc_s = cp.tile([128, 1536], f16, tag="enc", name="enc")
            nc.sync.dma_start(out=enc_s[:], in_=enc_d)

            def pixload(m, chunked=False):
                t = pixp.tile([128, 6144], f16, tag="pix", name="pix")
                if chunked:
                    # 12 small (j,pl)-chunk DMAs: the first S1 matmuls of the
                    # kernel can start after ~3 chunks instead of the full
                    # image, pulling the whole pipeline head in by ~5us
                    for j in range(4):
                        for pl in range(3):
                            nc.sync.dma_start(
                                out=t[:].rearrange("p (pl i c) -> p pl i c",
                                                   pl=3, i=4)
                                [:, pl, :, 128 * j:128 * (j + 1)],
                                in_=x_d[m, pl, :, 128 * j:128 * (j + 1)]
                                .rearrange("(i p) c -> p i c", i=4, p=128))
                else:
                    nc.sync.dma_start(
                        out=t[:].rearrange("p (pl i c) -> p pl i c", pl=3, i=4),
                        in_=x_d[m].rearrange("pl (i p) c -> p pl i c",
                                             i=4, p=128))
                return t

            fblk_s = cp.tile([128, 17], f32, tag="fblk", name="fblk")
            nc.sync.dma_start(out=fblk_s[:], in_=fblk_d)
            pix0 = pixload(0, chunked=True)
            pix1 = pixload(1)
            dec_s = cp.tile([128, 4480], f16, tag="dec", name="dec")
            nc.sync.dma_start(out=dec_s[:], in_=dec_d)

            w1_s = [enc_s[:, 256 * p:256 * (p + 1)] for p in range(3)]
            w2y_s = [enc_s[:, 768 + 128 * m:768 + 128 * (m + 1)]
                     for m in range(IMGS)]
            w2c_s = [enc_s[:, 1280 + 64 * m:1280 + 64 * (m + 1)]
                     for m in range(IMGS)]
            w3y_s = [dec_s[:, 128 * m:128 * (m + 1)] for m in range(IMGS)]
            w3c_s = [dec_s[:, 512 + 64 * m:512 + 64 * (m + 1)]
                     for m in range(IMGS)]
            w4y_s = dec_s[:, 768:896]
            w4cs_s = [dec_s[:, 896 + 128 * k:896 + 128 * (k + 1)]
                      for k in range(4)]
            tfull_s = dec_s[:, 1408:4480]
            rho_y_s = fblk_s[:, 0:8]
            rho_c_s = fblk_s[:, 8:16]
            zyb_s = fblk_s[:, 16:17]

            def bcast8(t):  # [128,8] const -> [128,64,8] step-0 broadcast (==512)
                return t[:, None, :].broadcast_to((128, 64, 8))

            def bcast8w(t):  # wide variant (==1024)
                return t[:, None, :].broadcast_to((128, 128, 8))

            def mm(out, lhsT, rhs, **kw):
                nc.tensor.matmul(out, lhsT=lhsT, rhs=rhs, **kw)

            def _front(m, pixw=None):
                """pix DMA + S1 + S2 + q-eviction for image m -> qw tile."""
                if pixw is None:
                    pixw = pixload(m)
                pix = [[pixw[:, 2048 * p + 512 * i:2048 * p + 512 * (i + 1)]
                        for i in range(4)] for p in range(3)]

                # ---- S1 (ds): vertical DCT (+color fold); 2 psum banks per
                # c-chunk, bank b: [i=2b: Y128 cb64 cr64 | i=2b+1: ...];
                # groups stay strictly sequential (interleaved open psum
                # accumulation groups mis-accumulate on HW), banks evict as
                # straight contiguous copies and S2 reads strided views ----
                h1 = []
                for j in range(4):
                    banks = [pp.tile([128, 512], f32, tag="psA", name="psS1",
                                     bufs=2) for _ in range(2)]
                    for i in range(4):
                        bank = banks[i // 2]
                        o0 = 256 * (i % 2)
                        for p in range(3):
                            mm(bank[:, o0:o0 + 256],
                               lhsT=pix[p][i][:, 128 * j:128 * (j + 1)],
                               rhs=w1_s[p],
                               start=(p == 0), stop=(p == 2))
                    th = h1p.tile([128, 1024], f16, tag="h1", name="h1")
                    for b in range(2):
                        nc.scalar.copy(out=th[:, 512 * b:512 * (b + 1)],
                                       in_=banks[b][:])
                    h1.append(th)
                # strided rhs views of h1[j] = [128, (i 4, [Y128 cb64 cr64])]
                h1v = [h1[j][:].rearrange("p (i s) -> p i s", i=4, s=256)
                       for j in range(4)]

                # ---- S2 (std): horizontal DCT -> merged coeff tile ----
                qw = qp.tile([128, 3072], f32, tag="q", name="q")
                for j in range(4):
                    psQ = pp.tile([128, 512], f32, tag="psB", name="psQ", bufs=2)
                    mm(psQ[:], lhsT=w2y_s[m], rhs=h1v[j][:, :, 0:128],
                       start=True, stop=True)
                    nc.vector.tensor_tensor(
                        out=qw[:, 512 * j:512 * (j + 1)], in0=psQ[:],
                        in1=bcast8(rho_y_s), op=op.mult)
                # chroma: one [128,512] psum per j-pair b; rows 0:64 = cb,
                # 64:128 = cr (partition-offset matmul writes)
                for b in range(2):
                    psQ = pp.tile([128, 512], f32, tag="psB", name="psQc", bufs=2)
                    for ch in range(2):
                        for jj in range(2):
                            j = 2 * b + jj
                            mm(psQ[64 * ch:64 * ch + 64,
                                   256 * jj:256 * (jj + 1)],
                               lhsT=w2c_s[m],
                               rhs=h1v[j][:, :, 128 + 64 * ch:192 + 64 * ch],
                               start=True, stop=True)
                    nc.vector.tensor_tensor(
                        out=qw[:, 2048 + 512 * b:2048 + 512 * (b + 1)],
                        in0=psQ[:], in1=bcast8(rho_c_s), op=op.mult)
                return qw

            def _qb(m, qw):
                """diff-round for image m: six wide passes -> r2 [128,3072]."""
                t1w = tp.tile([128, 3072], f16, tag="t1", name="t1", bufs=2)
                dpw = tp.tile([128, 3072], f16, tag="dp", name="dp", bufs=2)
                d2w = tp.tile([128, 3072], f16, tag="d2", name="d2", bufs=2)
                c3w = tp.tile([128, 3072], f16, tag="c3", name="c3")
                rw = tp.tile([128, 3072], f16, tag="r", name="r")
                r2w = r2p.tile([128, 3072], f16, tag="r2", name="r2")
                # t1 = f16(q + MAGIC) = MAGIC + round(q) exactly; 2x_2p
                nc.vector.tensor_scalar_add(out=t1w[:], in0=qw[:], scalar1=MAGIC)
                # dp = (t1 - MAGIC) - q = round(q) - q = -d   (1x: q is f32)
                nc.vector.scalar_tensor_tensor(
                    out=dpw[:], in0=t1w[:], scalar=-MAGIC, in1=qw[:],
                    op0=op.add, op1=op.subtract)
                # d2 = d^2; engine split across images balances ACT vs DVE
                if m in (0, 2):
                    nc.vector.tensor_tensor(out=d2w[:], in0=dpw[:], in1=dpw[:],
                                            op=op.mult)
                else:
                    nc.scalar.square(out=d2w[:], in_=dpw[:])
                # c3 = d2*dp = -d^3   (2x_1p)
                nc.vector.tensor_tensor(out=c3w[:], in0=d2w[:], in1=dpw[:],
                                        op=op.mult)
                # r = (t1 - MAGIC) - c3 = round(q) + d^3  (all-f16 -> 2x_1p)
                nc.vector.scalar_tensor_tensor(
                    out=rw[:], in0=t1w[:], scalar=-MAGIC, in1=c3w[:],
                    op0=op.add, op1=op.subtract)
                # r2 = r * T  (2x_1p vs materialized full-T tile)
                nc.vector.tensor_tensor(out=r2w[:], in0=rw[:], in1=tfull_s,
                                        op=op.mult)
                return r2w

            def _s34(m, r2w):
                """S3 + S4 + store for image m."""
                r2y = [r2w[:, 512 * j:512 * (j + 1)] for j in range(4)]

                # ---- S3 (ds): horizontal iDCT -> [(I,u), c] ----
                zy = []
                for i in range(4):
                    psZ = pp.tile([128, 512], f32, tag="psC", name="psZ", bufs=2)
                    for j in range(4):
                        mm(psZ[:, 128 * j:128 * (j + 1)],
                           lhsT=r2y[j][:, 128 * i:128 * (i + 1)],
                           rhs=w3y_s[m], start=True, stop=True)
                    t_ = zp.tile([128, 512], f16, tag="zy", name="zy")
                    # eviction carries the decode-side +128-on-Y as a
                    # per-partition bias on u==0 rows
                    nc.scalar.activation(out=t_[:], in_=psZ[:], func=AF.Identity,
                                         bias=zyb_s)
                    zy.append(t_)
                # chroma Z at HALF width (no horizontal upsample): per ch one
                # [128,512] psum packing (k=row-pair, j, 64 cols)
                zc = []
                for ch in range(2):
                    psZ = pp.tile([128, 512], f32, tag="psC", name="psZc", bufs=2)
                    po = 64 * ch
                    for k in range(2):
                        for j in range(4):
                            fo = 2048 + 512 * (j // 2) + 256 * (j % 2) + 128 * k
                            mm(psZ[:, 256 * k + 64 * j:256 * k + 64 * (j + 1)],
                               lhsT=r2w[po:po + 64, fo:fo + 128],
                               rhs=w3c_s[m][po:po + 64, :],
                               start=True, stop=True)
                    t_ = zp.tile([128, 512], f16, tag="zc", name="zc")
                    nc.scalar.copy(out=t_[:], in_=psZ[:])
                    zc.append(t_)

                # ---- S4 (std): vertical iDCT with YCC->RGB folded into the
                # PSUM accumulation; Relu-evict on ACT + min(x,1) on DVE ----
                # output layout (i, pl, c): the three Relu-evicted planes of
                # one column-chunk sit adjacent, so one 4x-mode min() clamps
                # all three at once
                outw = op_.tile([128, 6144], f16, tag="o", name="o")
                for i in range(4):
                    po = 64 * (i % 2)
                    k = i // 2

                    def cview(ch):  # [64,256] half-width -> [64,256,2] repeat
                        v = zc[ch][po:po + 64, 256 * k:256 * (k + 1)]
                        return v[:, :, None].broadcast_to((64, 256, 2))

                    zcb = cview(0)
                    zcr = cview(1)

                    ev3 = evp.tile([128, 1536], f16, tag="ev", name="ev")

                    def clamp(pl, ps):
                        nc.scalar.activation(
                            out=ev3[:, 512 * pl:512 * (pl + 1)], in_=ps[:],
                            func=AF.Relu)
                        if pl == 2:
                            nc.vector.tensor_scalar_min(
                                out=outw[:, 1536 * i:1536 * (i + 1)],
                                in0=ev3[:], scalar1=1.0)

                    psR = pp.tile([128, 512], f32, tag="psD", name="psR", bufs=2)
                    psG = pp.tile([128, 512], f32, tag="psD", name="psG", bufs=2)
                    mm(psR[:], lhsT=w4y_s, rhs=zy[i][:], start=True, stop=False)
                    mm(psR[:], lhsT=w4cs_s[0][po:po + 64, :], rhs=zcr,
                       start=False, stop=True)
                    mm(psG[:], lhsT=w4y_s, rhs=zy[i][:], start=True, stop=False)
                    mm(psG[:], lhsT=w4cs_s[1][po:po + 64, :], rhs=zcb,
                       start=False, stop=False)
                    clamp(0, psR)
                    mm(psG[:], lhsT=w4cs_s[2][po:po + 64, :], rhs=zcr,
                       start=False, stop=True)
                    psB = pp.tile([128, 512], f32, tag="psD", name="psB", bufs=2)
                    mm(psB[:], lhsT=w4y_s, rhs=zy[i][:], start=True, stop=False)
                    mm(psB[:], lhsT=w4cs_s[3][po:po + 64, :], rhs=zcb,
                       start=False, stop=True)
                    clamp(1, psG)
                    clamp(2, psB)
                for i in range(4):
                    nc.sync.dma_start(
                        out=out_d[m, :, 128 * i:128 * (i + 1), :].rearrange(
                            "pl p c -> p pl c"),
                        in_=outw[:, 1536 * i:1536 * (i + 1)].rearrange(
                            "p (pl c) -> p pl c", pl=3))

            def _build_images():
                # three-deep software pipeline: diff-round of image m is
                # emitted ahead of image m+2's front so its DVE/ACT work
                # overlaps two images' worth of PE time before S3(m) needs it
                qts = [None] * IMGS
                r2s = [None] * IMGS
                qts[0] = _front(0, pix0)
                qts[1] = _front(1, pix1)
                r2s[0] = _qb(0, qts[0])
                for m in range(IMGS):
                    if m + 2 < IMGS:
                        qts[m + 2] = _front(m + 2)
                    _s34(m, r2s[m])
                    r2s[m] = qts[m] = None
                    if m + 1 < IMGS:
                        r2s[m + 1] = _qb(m + 1, qts[m + 1])

            if repeat == 1:
                _build_images()
            else:
                with tc.For_i(0, repeat, 1):
                    _build_images()
    nc.compile()
    return nc


_NC_CACHE = {}


def _get_nc():
    if "nc" not in _NC_CACHE:
        _NC_CACHE["nc"] = build_program()
    return _NC_CACHE["nc"]


def kernel(x, quality):
    """Full inputs -> full output. Shards batch over 8 cores internally."""
    from concourse import bass_utils
    x = np.asarray(x, dtype=np.float32)
    quality = np.asarray(quality, dtype=np.float32)
    B = x.shape[0]
    assert B == N_CORES * IMGS, (B, N_CORES, IMGS)
    nc = _get_nc()
    in_maps = []
    for c in range(N_CORES):
        sl = slice(c * IMGS, (c + 1) * IMGS)
        in_maps.append(build_core_inputs(x[sl], quality[sl]))
    res = bass_utils.run_bass_kernel_spmd(nc, in_maps, core_ids=list(range(N_CORES)))
    outs = [np.asarray(res.results[c]["out"]).astype(np.float32)
            for c in range(N_CORES)]
    return np.concatenate(outs, axis=0)


# revision 23
# speedup vs baseline: 1.4503x; 1.4503x over previous
"""DiffJPEG Trainium2 Bass kernel (self-contained).

Pure data-parallel over 8 NeuronCores (4 images each). Per image the pipeline
is four matmul stages in a ds/std/ds/std chain (ds = data-stationary: image
data rides the PE stationary operand; std = constant-stationary weights, data
streams as a wide rhs):

  S1 ds  [row,col] -> [col,(I,u)] : vertical DCT (+RGB->YCC fold, 2x1 avg)
       Y and chroma accumulate into SEPARATE psum banks laid out so each
       evicts with ONE contiguous [128,512] ACT copy (no strided shuffles).
  S2 std -> [(J,v),(I,u)]         : horizontal DCT (+1/fq fold; 1x2 avg)
  quant: one [128,3072] f32 tile per image holds all Y+C coefficients; the
       diff-round runs as SIX wide passes (q, t1=q+MAGIC [DVE 2x_2p],
       dp [DVE 1x], d2 [ACT square], g [DVE 2x], r [DVE 1x], r2=r*T
       [DVE 2x_1p vs a materialized full T tile]).
  S3 ds  -> [(I,u),col]           : horizontal iDCT (+fq fold). Chroma stays
       at HALF width (no horizontal upsample here) - S4 reads it through a
       step-0 column-repeat AP, halving S3 chroma PE+eviction work.
  S4 std -> [(I,x),col]           : vertical iDCT + YCC->RGB folded into PSUM
       accumulation (chroma weights pre-scaled; vertical 2x upsample folded
       in w4cs). Output clamp is split: ACT evicts with Relu (lower bound),
       DVE applies min(x,1) in 4x mode on the fp16 tile.

Precision: encode side fp16 (bf16 flips quantizer rounding decisions);
decode side fp16 throughout (more mantissa than bf16; all decode values are
well inside fp16 range). Pixels are host-centered by -128/255 so color rows
annihilate the DC offset; decode +128 rides the S3 Y eviction as an ACT bias
on u==0 partitions. Images are two-deep software-pipelined so engines stay
busy across stage boundaries. GPSIMD/Pool deliberately unused (real-HW cost
far exceeds the model).
"""
import sys
import numpy as np

sys.path.insert(0, "/opt/trn_rl_repo")

import ml_dtypes

F16 = np.float16
N_CORES = 8
IMGS = 4          # images per core
H = W = 512
MAGIC = 1536.0  # 1.5*2**10: f16(x+M)-M == round-half-even(x) for |x|<512
# (|q| = |coeff/(T*fq)| <= ~320 here; the f32->f16 double rounding flips
# ~1e-4 of round decisions, adding ~3e-3 rel err - well inside tolerance,
# and it makes the whole correction chain 16-bit so DVE runs in 2x mode)

# ---------------------------------------------------------------------------
# host-side constants
# ---------------------------------------------------------------------------
_xs = np.arange(8, dtype=np.float32)
_COS = np.cos((2 * _xs[:, None] + 1) * _xs[None, :] * np.pi / 16).astype(np.float32)
_alpha = np.array([1.0 / np.sqrt(2)] + [1.0] * 7, dtype=np.float32)
_Y_TABLE = np.array([
    [16, 11, 10, 16, 24, 40, 51, 61], [12, 12, 14, 19, 26, 58, 60, 55],
    [14, 13, 16, 24, 40, 57, 69, 56], [14, 17, 22, 29, 51, 87, 80, 62],
    [18, 22, 37, 56, 68, 109, 103, 77], [24, 35, 55, 64, 81, 104, 113, 92],
    [49, 64, 78, 87, 103, 121, 120, 101], [72, 92, 95, 98, 112, 100, 103, 99]],
    dtype=np.float32)
_C_TABLE = np.full((8, 8), 99.0, dtype=np.float32)
_C_TABLE[:4, :4] = np.array([[17, 18, 24, 47], [18, 21, 26, 66],
                             [24, 26, 56, 99], [47, 66, 99, 99]], dtype=np.float32)
_RGB2YCC = np.array([[0.299, 0.587, 0.114],
                    [-0.168736, -0.331264, 0.5],
                    [0.5, -0.418688, -0.081312]], dtype=np.float32)
_YCC2RGB = np.array([[1.0, 0.0, 1.402],
                    [1.0, -0.344136, -0.714136],
                    [1.0, 1.772, 0.0]], dtype=np.float32)


def _bd(M, n):
    r, c = M.shape
    out = np.zeros((r * n, c * n), dtype=np.float64)
    for i in range(n):
        out[i * r:(i + 1) * r, i * c:(i + 1) * c] = M
    return out


def _base_mats():
    Av = (_COS.astype(np.float64) * 0.5 * _alpha.astype(np.float64)[None, :])  # [x,u]
    Avi = Av.T.copy()                                   # [u,x]
    Avs = np.zeros((16, 8))                             # subsample fwd
    for x2 in range(16):
        Avs[x2] = Av[x2 // 2] / 2.0
    Avu = np.zeros((8, 16))                             # upsample inv
    for x2 in range(16):
        Avu[:, x2] = Avi[:, x2 // 2]
    return Av, Avi, Avs, Avu


def build_core_inputs(x_core, quality_core):
    """x_core [IMGS,3,512,512] f32, quality_core [IMGS] f32 -> in_map dict."""
    Av, Avi, Avs, Avu = _base_mats()
    f32 = np.float32
    bd16v = _bd(Av, 16)        # [128,128] 1D fwd (vertical or horizontal)
    bd8s = _bd(Avs, 8)         # [128,64]  fwd subsampled
    bd16i = _bd(Avi, 16)       # [128,128] 1D inverse
    bd8i = _bd(Avi, 8)         # [64,64]   inverse (no upsample)
    bd8u = _bd(Avu, 8)         # [64,128]  inverse upsampling
    bd8i2 = np.concatenate([bd8i, bd8i], axis=0)        # [128,64] parity-stacked
    bd8u2 = np.concatenate([bd8u, bd8u], axis=0)        # [128,128] parity-stacked

    # S1 rhs per plane: out cols = [Y-Iu 128 | cb-I'u 64 | cr-I'u 64]
    w1 = np.stack([
        np.concatenate([255.0 * _RGB2YCC[0, p] * bd16v,
                        255.0 * _RGB2YCC[1, p] * bd8s,
                        255.0 * _RGB2YCC[2, p] * bd8s], axis=1).astype(F16)
        for p in range(3)])                                            # [3,128,256]

    fqs = []
    for q in np.asarray(quality_core, dtype=np.float64):
        factor = (5000.0 / q if q < 50.0 else 200.0 - 2.0 * q) / 100.0
        fqs.append(factor)

    w2y = np.stack([(bd16v / fq).astype(F16) for fq in fqs])   # [4,128,128]
    w2c = np.stack([(bd8s / fq).astype(F16) for fq in fqs])    # [4,128,64]
    w3y = np.stack([(bd16i * fq).astype(F16) for fq in fqs])   # [4,128,128]
    w3c = np.stack([(bd8i2 * fq).astype(F16) for fq in fqs])   # [4,128,64]
    w4y = (bd16i / 255.0).astype(F16)                          # [128,128]
    C = _YCC2RGB.astype(np.float64)
    w4cs = np.stack([(c * bd8u2 / 255.0).astype(F16)
                     for c in (C[0, 2], C[1, 1], C[1, 2], C[2, 1])])   # [4,128,128]

    # quant patterns in [(J,v) partition, (I,u) free] layout:
    # value[p, f] = T[u(f%8), v(p%8)] -> tile T.T along partitions
    rho_y = np.tile((1.0 / _Y_TABLE).T, (16, 1)).astype(f32)           # [128,8]
    rho_c = np.tile((1.0 / _C_TABLE).T, (16, 1)).astype(f32)
    # full dequant-table tile matching the merged [128,3072] coeff layout
    tfull = np.concatenate(
        [np.tile(_Y_TABLE.T, (16, 256)), np.tile(_C_TABLE.T, (16, 128))],
        axis=1).astype(F16)                                            # [128,3072]

    mask = (np.arange(128) % 8 == 0).astype(f32)[:, None]
    # decode-side +128 on Y: bias on zy u==0 partitions through w4y's
    # Avi[0,x]/255 gain -> +0.5 on every output pixel
    zyb = (mask * (0.5 * 255.0 / float(Avi[0, 0]))).astype(f32)        # [128,1]

    # batched const blocks (one DMA each; per-DMA issue cost is ~500ns so 29
    # separate weight loads would stall the head of the kernel)
    enc = np.concatenate(
        [w1[0], w1[1], w1[2]] + [w2y[m] for m in range(IMGS)]
        + [w2c[m] for m in range(IMGS)], axis=1)                       # [128,1536]
    dec = np.concatenate(
        [w3y[m] for m in range(IMGS)] + [w3c[m] for m in range(IMGS)]
        + [w4y] + [w4cs[k] for k in range(4)] + [tfull], axis=1)       # [128,4480]
    fblk = np.concatenate([rho_y, rho_c, zyb], axis=1).astype(f32)     # [128,17]

    # centered pixels: the color rows annihilate the 128 offset exactly
    # (chroma rows sum to 0, Y row to 1), shrinking all encode magnitudes
    xc = np.ascontiguousarray(x_core, dtype=np.float32) - np.float32(128.0 / 255.0)
    return {
        "x": xc.astype(F16),
        "enc": np.ascontiguousarray(enc), "dec": np.ascontiguousarray(dec),
        "fblk": fblk,
    }


# ---------------------------------------------------------------------------
# bass program
# ---------------------------------------------------------------------------
def build_program(repeat=1):
    import concourse.bacc as bacc
    import concourse.mybir as mybir
    from concourse.tile import TileContext

    f32 = mybir.dt.float32
    f16 = mybir.dt.float16
    op = mybir.AluOpType
    AF = mybir.ActivationFunctionType

    nc = bacc.Bacc("TRN2", target_bir_lowering=False, debug=False,
                   enable_asserts=False, num_devices=N_CORES)

    x_d = nc.dram_tensor("x", [IMGS, 3, H, W], f16, kind="ExternalInput").ap()
    out_d = nc.dram_tensor("out", [IMGS, 3, H, W], f16, kind="ExternalOutput").ap()
    enc_d = nc.dram_tensor("enc", [128, 1536], f16, kind="ExternalInput").ap()
    dec_d = nc.dram_tensor("dec", [128, 4480], f16, kind="ExternalInput").ap()
    fblk_d = nc.dram_tensor("fblk", [128, 17], f32, kind="ExternalInput").ap()

    with TileContext(nc, trace_sim=False) as tc:
        with tc.tile_pool(name="consts", bufs=1) as cp, \
             tc.tile_pool(name="pix", bufs=3) as pixp, \
             tc.tile_pool(name="h1", bufs=8) as h1p, \
             tc.tile_pool(name="qq", bufs=3) as qp, \
             tc.tile_pool(name="tmp", bufs=1) as tp, \
             tc.tile_pool(name="r2", bufs=2) as r2p, \
             tc.tile_pool(name="zz", bufs=7) as zp, \
             tc.tile_pool(name="ev", bufs=4) as evp, \
             tc.tile_pool(name="outp", bufs=2) as op_, \
             tc.tile_pool(name="ps", bufs=1, space="PSUM") as pp:

            # ---- batched const loads, interleaved with the first two pixel
            # DMAs so S1 of image 0 can start ~6us in instead of ~19us ----
            enc_s = cp.tile([128, 1536], f16, tag="enc", name="enc")
            nc.sync.dma_start(out=enc_s[:], in_=enc_d)

            def pixload(m, chunked=False):
                t = pixp.tile([128, 6144], f16, tag="pix", name="pix")
                if chunked:
                    # 12 small (j,pl)-chunk DMAs: the first S1 matmuls of the
                    # kernel can start after ~3 chunks instead of the full
                    # image, pulling the whole pipeline head in by ~5us
                    for j in range(4):
                        for pl in range(3):
                            nc.sync.dma_start(
                                out=t[:].rearrange("p (pl i c) -> p pl i c",
                                                   pl=3, i=4)
                                [:, pl, :, 128 * j:128 * (j + 1)],
                                in_=x_d[m, pl, :, 128 * j:128 * (j + 1)]
                                .rearrange("(i p) c -> p i c", i=4, p=128))
                else:
                    nc.sync.dma_start(
                        out=t[:].rearrange("p (pl i c) -> p pl i c", pl=3, i=4),
                        in_=x_d[m].rearrange("pl (i p) c -> p pl i c",
                                             i=4, p=128))
                return t

            fblk_s = cp.tile([128, 17], f32, tag="fblk", name="fblk")
            nc.sync.dma_start(out=fblk_s[:], in_=fblk_d)
            pix0 = pixload(0, chunked=True)
            pix1 = pixload(1)
            dec_s = cp.tile([128, 4480], f16, tag="dec", name="dec")
            nc.sync.dma_start(out=dec_s[:], in_=dec_d)

            w1_s = [enc_s[:, 256 * p:256 * (p + 1)] for p in range(3)]
            w2y_s = [enc_s[:, 768 + 128 * m:768 + 128 * (m + 1)]
                     for m in range(IMGS)]
            w2c_s = [enc_s[:, 1280 + 64 * m:1280 + 64 * (m + 1)]
                     for m in range(IMGS)]
            w3y_s = [dec_s[:, 128 * m:128 * (m + 1)] for m in range(IMGS)]
            w3c_s = [dec_s[:, 512 + 64 * m:512 + 64 * (m + 1)]
                     for m in range(IMGS)]
            w4y_s = dec_s[:, 768:896]
            w4cs_s = [dec_s[:, 896 + 128 * k:896 + 128 * (k + 1)]
                      for k in range(4)]
            tfull_s = dec_s[:, 1408:4480]
            rho_y_s = fblk_s[:, 0:8]
            rho_c_s = fblk_s[:, 8:16]
            zyb_s = fblk_s[:, 16:17]

            def bcast8(t):  # [128,8] const -> [128,64,8] step-0 broadcast (==512)
                return t[:, None, :].broadcast_to((128, 64, 8))

            def bcast8w(t):  # wide variant (==1024)
                return t[:, None, :].broadcast_to((128, 128, 8))

            def mm(out, lhsT, rhs, **kw):
                nc.tensor.matmul(out, lhsT=lhsT, rhs=rhs, **kw)

            def _front(m, pixw=None):
                """pix DMA + S1 + S2 + q-eviction for image m -> qw tile."""
                if pixw is None:
                    pixw = pixload(m)
                pix = [[pixw[:, 2048 * p + 512 * i:2048 * p + 512 * (i + 1)]
                        for i in range(4)] for p in range(3)]

                # ---- S1 (ds): vertical DCT (+color fold); 2 psum banks per
                # c-chunk, bank b: [i=2b: Y128 cb64 cr64 | i=2b+1: ...];
                # groups stay strictly sequential (interleaved open psum
                # accumulation groups mis-accumulate on HW), banks evict as
                # straight contiguous copies and S2 reads strided views ----
                h1 = []
                for j in range(4):
                    banks = [pp.tile([128, 512], f32, tag="psA", name="psS1",
                                     bufs=2) for _ in range(2)]
                    for i in range(4):
                        bank = banks[i // 2]
                        o0 = 256 * (i % 2)
                        for p in range(3):
                            mm(bank[:, o0:o0 + 256],
                               lhsT=pix[p][i][:, 128 * j:128 * (j + 1)],
                               rhs=w1_s[p],
                               start=(p == 0), stop=(p == 2))
                    th = h1p.tile([128, 1024], f16, tag="h1", name="h1")
                    for b in range(2):
                        nc.scalar.copy(out=th[:, 512 * b:512 * (b + 1)],
                                       in_=banks[b][:])
                    h1.append(th)
                # strided rhs views of h1[j] = [128, (i 4, [Y128 cb64 cr64])]
                h1v = [h1[j][:].rearrange("p (i s) -> p i s", i=4, s=256)
                       for j in range(4)]

                # ---- S2 (std): horizontal DCT -> merged coeff tile ----
                qw = qp.tile([128, 3072], f32, tag="q", name="q")
                for j in range(4):
                    psQ = pp.tile([128, 512], f32, tag="psB", name="psQ", bufs=2)
                    mm(psQ[:], lhsT=w2y_s[m], rhs=h1v[j][:, :, 0:128],
                       start=True, stop=True)
                    nc.vector.tensor_tensor(
                        out=qw[:, 512 * j:512 * (j + 1)], in0=psQ[:],
                        in1=bcast8(rho_y_s), op=op.mult)
                # chroma: one [128,512] psum per j-pair b; rows 0:64 = cb,
                # 64:128 = cr (partition-offset matmul writes)
                for b in range(2):
                    psQ = pp.tile([128, 512], f32, tag="psB", name="psQc", bufs=2)
                    for ch in range(2):
                        for jj in range(2):
                            j = 2 * b + jj
                            mm(psQ[64 * ch:64 * ch + 64,
                                   256 * jj:256 * (jj + 1)],
                               lhsT=w2c_s[m],
                               rhs=h1v[j][:, :, 128 + 64 * ch:192 + 64 * ch],
                               start=True, stop=True)
                    nc.vector.tensor_tensor(
                        out=qw[:, 2048 + 512 * b:2048 + 512 * (b + 1)],
                        in0=psQ[:], in1=bcast8(rho_c_s), op=op.mult)
                return qw

            def _qb(m, qw):
                """diff-round for image m: six wide passes -> r2 [128,3072]."""
                t1w = tp.tile([128, 3072], f16, tag="t1", name="t1")
                dpw = tp.tile([128, 3072], f16, tag="dp", name="dp")
                d2w = tp.tile([128, 3072], f16, tag="d2", name="d2")
                c3w = tp.tile([128, 3072], f16, tag="c3", name="c3")
                rw = tp.tile([128, 3072], f16, tag="r", name="r")
                r2w = r2p.tile([128, 3072], f16, tag="r2", name="r2")
                # t1 = f16(q + MAGIC) = MAGIC + round(q) exactly; 2x_2p
                nc.vector.tensor_scalar_add(out=t1w[:], in0=qw[:], scalar1=MAGIC)
                # dp = (t1 - MAGIC) - q = round(q) - q = -d   (1x: q is f32)
                nc.vector.scalar_tensor_tensor(
                    out=dpw[:], in0=t1w[:], scalar=-MAGIC, in1=qw[:],
                    op0=op.add, op1=op.subtract)
                # d2 = d^2; engine split across images balances ACT vs DVE
                if m in (0, 2):
                    nc.vector.tensor_tensor(out=d2w[:], in0=dpw[:], in1=dpw[:],
                                            op=op.mult)
                else:
                    nc.scalar.square(out=d2w[:], in_=dpw[:])
                # c3 = d2*dp = -d^3   (2x_1p)
                nc.vector.tensor_tensor(out=c3w[:], in0=d2w[:], in1=dpw[:],
                                        op=op.mult)
                # r = (t1 - MAGIC) - c3 = round(q) + d^3  (all-f16 -> 2x_1p)
                nc.vector.scalar_tensor_tensor(
                    out=rw[:], in0=t1w[:], scalar=-MAGIC, in1=c3w[:],
                    op0=op.add, op1=op.subtract)
                # r2 = r * T  (2x_1p vs materialized full-T tile)
                nc.vector.tensor_tensor(out=r2w[:], in0=rw[:], in1=tfull_s,
                                        op=op.mult)
                return r2w

            def _s34(m, r2w):
                """S3 + S4 + store for image m."""
                r2y = [r2w[:, 512 * j:512 * (j + 1)] for j in range(4)]

                # ---- S3 (ds): horizontal iDCT -> [(I,u), c] ----
                zy = []
                for i in range(4):
                    psZ = pp.tile([128, 512], f32, tag="psC", name="psZ", bufs=2)
                    for j in range(4):
                        mm(psZ[:, 128 * j:128 * (j + 1)],
                           lhsT=r2y[j][:, 128 * i:128 * (i + 1)],
                           rhs=w3y_s[m], start=True, stop=True)
                    t_ = zp.tile([128, 512], f16, tag="zy", name="zy")
                    # eviction carries the decode-side +128-on-Y as a
                    # per-partition bias on u==0 rows
                    nc.scalar.activation(out=t_[:], in_=psZ[:], func=AF.Identity,
                                         bias=zyb_s)
                    zy.append(t_)
                # chroma Z at HALF width (no horizontal upsample): per ch one
                # [128,512] psum packing (k=row-pair, j, 64 cols)
                zc = []
                for ch in range(2):
                    psZ = pp.tile([128, 512], f32, tag="psC", name="psZc", bufs=2)
                    po = 64 * ch
                    for k in range(2):
                        for j in range(4):
                            fo = 2048 + 512 * (j // 2) + 256 * (j % 2) + 128 * k
                            mm(psZ[:, 256 * k + 64 * j:256 * k + 64 * (j + 1)],
                               lhsT=r2w[po:po + 64, fo:fo + 128],
                               rhs=w3c_s[m][po:po + 64, :],
                               start=True, stop=True)
                    t_ = zp.tile([128, 512], f16, tag="zc", name="zc")
                    nc.scalar.copy(out=t_[:], in_=psZ[:])
                    zc.append(t_)

                # ---- S4 (std): vertical iDCT with YCC->RGB folded into the
                # PSUM accumulation; Relu-evict on ACT + min(x,1) on DVE ----
                # output layout (i, pl, c): the three Relu-evicted planes of
                # one column-chunk sit adjacent, so one 4x-mode min() clamps
                # all three at once
                outw = op_.tile([128, 6144], f16, tag="o", name="o")
                for i in range(4):
                    po = 64 * (i % 2)
                    k = i // 2

                    def cview(ch):  # [64,256] half-width -> [64,256,2] repeat
                        v = zc[ch][po:po + 64, 256 * k:256 * (k + 1)]
                        return v[:, :, None].broadcast_to((64, 256, 2))

                    zcb = cview(0)
                    zcr = cview(1)

                    ev3 = evp.tile([128, 1536], f16, tag="ev", name="ev")

                    def clamp(pl, ps):
                        nc.scalar.activation(
                            out=ev3[:, 512 * pl:512 * (pl + 1)], in_=ps[:],
                            func=AF.Relu)
                        if pl == 2:
                            nc.vector.tensor_scalar_min(
                                out=outw[:, 1536 * i:1536 * (i + 1)],
                                in0=ev3[:], scalar1=1.0)

                    psR = pp.tile([128, 512], f32, tag="psD", name="psR", bufs=2)
                    psG = pp.tile([128, 512], f32, tag="psD", name="psG", bufs=2)
                    mm(psR[:], lhsT=w4y_s, rhs=zy[i][:], start=True, stop=False)
                    mm(psR[:], lhsT=w4cs_s[0][po:po + 64, :], rhs=zcr,
                       start=False, stop=True)
                    mm(psG[:], lhsT=w4y_s, rhs=zy[i][:], start=True, stop=False)
                    mm(psG[:], lhsT=w4cs_s[1][po:po + 64, :], rhs=zcb,
                       start=False, stop=False)
                    clamp(0, psR)
                    mm(psG[:], lhsT=w4cs_s[2][po:po + 64, :], rhs=zcr,
                       start=False, stop=True)
                    psB = pp.tile([128, 512], f32, tag="psD", name="psB", bufs=2)
                    mm(psB[:], lhsT=w4y_s, rhs=zy[i][:], start=True, stop=False)
                    mm(psB[:], lhsT=w4cs_s[3][po:po + 64, :], rhs=zcb,
                       start=False, stop=True)
                    clamp(1, psG)
                    clamp(2, psB)
                for i in range(4):
                    nc.sync.dma_start(
                        out=out_d[m, :, 128 * i:128 * (i + 1), :].rearrange(
                            "pl p c -> p pl c"),
                        in_=outw[:, 1536 * i:1536 * (i + 1)].rearrange(
                            "p (pl c) -> p pl c", pl=3))

            def _build_images():
                # three-deep software pipeline: diff-round of image m is
                # emitted ahead of image m+2's front so its DVE/ACT work
                # overlaps two images' worth of PE time before S3(m) needs it
                qts = [None] * IMGS
                r2s = [None] * IMGS
                qts[0] = _front(0, pix0)
                qts[1] = _front(1, pix1)
                r2s[0] = _qb(0, qts[0])
                for m in range(IMGS):
                    if m + 2 < IMGS:
                        qts[m + 2] = _front(m + 2)
                    _s34(m, r2s[m])
                    r2s[m] = qts[m] = None
                    if m + 1 < IMGS:
                        r2s[m + 1] = _qb(m + 1, qts[m + 1])

            if repeat == 1:
                _build_images()
            else:
                with tc.For_i(0, repeat, 1):
                    _build_images()
    nc.compile()
    return nc


_NC_CACHE = {}


def _get_nc():
    if "nc" not in _NC_CACHE:
        _NC_CACHE["nc"] = build_program()
    return _NC_CACHE["nc"]


def kernel(x, quality):
    """Full inputs -> full output. Shards batch over 8 cores internally."""
    from concourse import bass_utils
    x = np.asarray(x, dtype=np.float32)
    quality = np.asarray(quality, dtype=np.float32)
    B = x.shape[0]
    assert B == N_CORES * IMGS, (B, N_CORES, IMGS)
    nc = _get_nc()
    in_maps = []
    for c in range(N_CORES):
        sl = slice(c * IMGS, (c + 1) * IMGS)
        in_maps.append(build_core_inputs(x[sl], quality[sl]))
    res = bass_utils.run_bass_kernel_spmd(nc, in_maps, core_ids=list(range(N_CORES)))
    outs = [np.asarray(res.results[c]["out"]).astype(np.float32)
            for c in range(N_CORES)]
    return np.concatenate(outs, axis=0)
